# revision 1
# baseline (speedup 1.0000x reference)
"""Gemma3n text attention on 8 Trainium2 NeuronCores (Bass/Tile).

Sharding: core c = b*4 + kv*2 + qp handles batch b, KV head kv and the
q-head pair (kv*4 + qp*2, kv*4 + qp*2 + 1).  Each core computes the
Q/K/V projections for its shard, QK-norm + RoPE, causal attention for
its two query heads, and a partial output projection against its
512-column slice of Wo.  The host sums the four partials per batch.

Self-contained: only needs numpy + the concourse tree that ships in the
container image (on PYTHONPATH at /root/.axon_site/_ro/trn_rl_repo).
"""

import sys

for _p in ("/root/.axon_site/_ro/trn_rl_repo", "/opt/trn_rl_repo"):
    if _p not in sys.path:
        sys.path.append(_p)

from contextlib import ExitStack

import numpy as np

import concourse.bass as bass
import concourse.mybir as mybir
import concourse.tile as tile
from concourse import bacc
from concourse.masks import make_identity

P = 128
B, S, HID = 2, 2048, 2048
NH, NKV, HD = 8, 2, 256
DQ = 2 * HD            # q-width per core (2 heads)
NSC = S // P           # 16 seq chunks
NHC = HID // P         # 16 hidden chunks
EPS = 1e-6

f32 = mybir.dt.float32
f32r = mybir.dt.float32r
i32 = mybir.dt.int32
FMIN = float(np.finfo(np.float32).min)
ACT = mybir.ActivationFunctionType


def to_f32r(arr):
    """Round fp32 -> fp32r bit format (11 explicit mantissa bits, RNE).

    Bit-exact with libwalrus fp32_to_fp32r."""
    u = np.ascontiguousarray(arr, np.float32).view(np.uint32)
    r = ((u.astype(np.uint64) + 0x7FF + ((u >> 12) & 1)) & 0xFFFFF000)
    return r.astype(np.uint32).view(np.float32)


def build_program(use_f32r=True, use_tmr=False):
    """Emit the SPMD per-core program. Returns the compiled Bacc object."""
    nc = bacc.Bacc("TRN2", target_bir_lowering=False, debug=False, num_devices=8)

    mdt = f32r if use_f32r else f32   # dtype of every matmul operand

    hT_d = nc.dram_tensor("hT", [NHC, P, S], mdt, kind="ExternalInput")
    wT_d = nc.dram_tensor("wT", [NHC, P, DQ + 2 * HD], mdt, kind="ExternalInput")
    csq_d = nc.dram_tensor("csq", [NSC, P, 2 * HD], f32, kind="ExternalInput")
    csk_d = nc.dram_tensor("csk", [NSC, P, 2 * HD], f32, kind="ExternalInput")
    woT_d = nc.dram_tensor("woT", [4, P, HID], mdt, kind="ExternalInput")
    out_d = nc.dram_tensor("out", [S, HID], f32, kind="ExternalOutput")

    with tile.TileContext(nc) as tc, ExitStack() as ctx:
        const = ctx.enter_context(tc.tile_pool(name="const", bufs=1))
        persist = ctx.enter_context(tc.tile_pool(name="persist", bufs=1))

        ident = const.tile([P, P], f32)
        make_identity(nc, ident)
        mdiag = const.tile([P, P], f32)      # 0 on/below diag, -1e9 above
        nc.gpsimd.memset(mdiag, 0.0)
        nc.gpsimd.affine_select(out=mdiag, in_=mdiag,
                                compare_op=mybir.AluOpType.is_ge, fill=-1e9,
                                base=0, pattern=[[-1, P]], channel_multiplier=1)
        eps_t = const.tile([P, 1], f32)
        nc.vector.memset(eps_t, EPS)

        # persistent SBUF tensors (qT/kT/v: 64KB per partition)
        qT = persist.tile([P, 2, 2, S], mdt)      # [d, head, dchunk, qpos]
        kT = persist.tile([P, 2, S], mdt)         # [d, dchunk, kpos]
        v_sb = persist.tile([P, NSC, HD], mdt)    # [kpos, kchunk, d]
        rq_all = persist.tile([P, NSC, 2], f32)   # per-row q rstd (folded in exp)

        # ------- Phase A: QKV proj + norm + rope + transposes (fused) --------
        with ExitStack() as a1:
            hpool = a1.enter_context(tc.tile_pool(name="hTp", bufs=3))
            wpool = a1.enter_context(tc.tile_pool(name="wTp", bufs=1))
            wt_all = wpool.tile([P, NHC, DQ + 2 * HD], mdt)
            nc.sync.dma_start(wt_all, wT_d.ap().rearrange("h p d -> p h d"))
            cpool = a1.enter_context(tc.tile_pool(name="cs", bufs=3))
            epool = a1.enter_context(tc.tile_pool(name="evict", bufs=4))
            spool = a1.enter_context(tc.tile_pool(name="small", bufs=8))
            psA = a1.enter_context(tc.tile_pool(name="psA", bufs=6, space="PSUM"))
            psT = a1.enter_context(tc.tile_pool(name="psT", bufs=2, space="PSUM"))

            groups = [2] * 8                  # 4 banks/group; 6-buf pool overlaps
            sc0 = 0
            for g, gn in enumerate(groups):
                psq = [psA.tile([P, DQ], f32, tag="ps", name=f"psq{g}_{jj}")
                       for jj in range(gn)]
                pskv = [psA.tile([P, 2 * HD], f32, tag="ps", name=f"pskv{g}_{jj}")
                        for jj in range(gn)]
                for hc in range(NHC):
                    th = hpool.tile([P, gn * P], mdt, tag="h")
                    nc.sync.dma_start(th, hT_d[hc, :, sc0 * P:(sc0 + gn) * P])
                    tw = wt_all[:, hc]
                    st, sp = hc == 0, hc == NHC - 1
                    for j in range(gn):
                        lhs = th[:, j * P:(j + 1) * P]
                        nc.tensor.matmul(psq[j][:], lhs, tw[:, 0:DQ],
                                         start=st, stop=sp)
                        nc.tensor.matmul(pskv[j][:], lhs, tw[:, DQ:],
                                         start=st, stop=sp)
                for j in range(gn):
                    sc = sc0 + j
                    # sum of squares per 256-group via ACT Square (reads PSUM)
                    ssq = spool.tile([P, 4], f32, tag="ssq")
                    scr = epool.tile([P, HD], f32, tag="scr")
                    nc.scalar.activation(scr[:], psq[j][:, 0:HD], ACT.Square,
                                         accum_out=ssq[:, 0:1])
                    nc.scalar.activation(scr[:], psq[j][:, HD:2 * HD],
                                         ACT.Square, accum_out=ssq[:, 1:2])
                    nc.scalar.activation(scr[:], pskv[j][:, 0:HD], ACT.Square,
                                         accum_out=ssq[:, 2:3])
                    nc.scalar.activation(scr[:], pskv[j][:, HD:2 * HD],
                                         ACT.Square, accum_out=ssq[:, 3:4])
                    rstd = spool.tile([P, 4], f32, tag="rstd")
                    nc.scalar.activation(rstd[:], ssq[:], ACT.Sqrt,
                                         bias=eps_t[:], scale=1.0 / HD)
                    nc.vector.reciprocal(rq_all[:, sc, :], rstd[:, 0:2])
                    nc.vector.reciprocal(rstd[:, 2:4], rstd[:, 2:4])

                    # v: scale + evict in one DVE op
                    nc.vector.tensor_scalar_mul(out=v_sb[:, sc, :],
                                                in0=pskv[j][:, HD:2 * HD],
                                                scalar1=rstd[:, 3:4])

                    csq = cpool.tile([P, 2 * HD], f32, tag="csq")
                    nc.sync.dma_start(csq, csq_d[sc])
                    csk = cpool.tile([P, 2 * HD], f32, tag="csk")
                    nc.sync.dma_start(csk, csk_d[sc])

                    # rope(x) = x*cosw + swap(x)*sinw (sinw lo pre-negated);
                    # reads projection PSUM directly, writes SBUF
                    qro = epool.tile([P, DQ], f32, tag="qro")
                    kro = epool.tile([P, HD], f32, tag="kro")
                    for h in range(2):
                        b0 = h * HD
                        tmp = epool.tile([P, HD], f32, tag="tmp")
                        nc.vector.tensor_mul(tmp[:, 0:P],
                                             psq[j][:, b0 + P:b0 + HD],
                                             csq[:, HD:HD + P])
                        nc.vector.tensor_mul(tmp[:, P:HD],
                                             psq[j][:, b0:b0 + P],
                                             csq[:, HD + P:2 * HD])
                        qh = qro[:, b0:b0 + HD]
                        nc.vector.tensor_mul(qh, psq[j][:, b0:b0 + HD],
                                             csq[:, 0:HD])
                        nc.vector.tensor_add(qh, qh, tmp[:])
                    tmp = epool.tile([P, HD], f32, tag="tmp")
                    nc.vector.tensor_mul(tmp[:, 0:P], pskv[j][:, P:HD],
                                         csk[:, HD:HD + P])
                    nc.vector.tensor_mul(tmp[:, P:HD], pskv[j][:, 0:P],
                                         csk[:, HD + P:2 * HD])
                    nc.vector.tensor_mul(kro[:], pskv[j][:, 0:HD], csk[:, 0:HD])
                    nc.vector.tensor_add(kro[:], kro[:], tmp[:])
                    nc.vector.tensor_scalar_mul(out=kro[:], in0=kro[:],
                                                scalar1=rstd[:, 2:3])

                    # transposes into qT/kT (PE); paired evictions
                    for h in range(2):
                        pt = psT.tile([P, 2 * P], f32, tag="t")
                        for dc in range(2):
                            nc.tensor.transpose(
                                pt[:, dc * P:(dc + 1) * P],
                                qro[:, h * HD + dc * P:h * HD + (dc + 1) * P],
                                ident[:])
                        dst = qT[:, h, 0:2, sc * P:(sc + 1) * P]
                        if (sc + h) % 2 == 0:
                            nc.scalar.copy(dst, pt[:].rearrange(
                                "p (a b) -> p a b", a=2))
                        else:
                            nc.vector.tensor_copy(out=dst, in_=pt[:].rearrange(
                                "p (a b) -> p a b", a=2))
                    pt = psT.tile([P, 2 * P], f32, tag="t")
                    for dc in range(2):
                        nc.tensor.transpose(pt[:, dc * P:(dc + 1) * P],
                                            kro[:, dc * P:(dc + 1) * P],
                                            ident[:])
                    dst = kT[:, 0:2, sc * P:(sc + 1) * P]
                    if sc % 2 == 0:
                        nc.vector.tensor_copy(out=dst, in_=pt[:].rearrange(
                            "p (a b) -> p a b", a=2))
                    else:
                        nc.scalar.copy(dst, pt[:].rearrange(
                            "p (a b) -> p a b", a=2))
                sc0 += gn

        # ---------------- Phase B: attention per (head, q-block) -------------
        wopool = ctx.enter_context(tc.tile_pool(name="wo", bufs=1))
        woT = wopool.tile([P, 4, HID], mdt)
        for t in range(4):
            nc.sync.dma_start(woT[:, t, :], woT_d[t])
        atpool = ctx.enter_context(tc.tile_pool(name="attnT", bufs=1))
        attnT = atpool.tile([P, 4, S], mdt)       # [d2, (h,dc), qpos]

        with ExitStack() as bctx:
            pss = bctx.enter_context(tc.tile_pool(name="pss", bufs=2, space="PSUM"))
            pst = bctx.enter_context(tc.tile_pool(name="pst", bufs=2, space="PSUM"))
            psv = bctx.enter_context(tc.tile_pool(name="psv", bufs=1, space="PSUM"))
            ppool = bctx.enter_context(tc.tile_pool(name="prp", bufs=2))
            tpool = bctx.enter_context(tc.tile_pool(name="ptsp", bufs=6))
            apool = bctx.enter_context(tc.tile_pool(name="attnp", bufs=2))
            dpool = bctx.enter_context(tc.tile_pool(name="denp", bufs=8))
            pso = bctx.enter_context(tc.tile_pool(name="pso", bufs=1, space="PSUM"))
            opool = bctx.enter_context(tc.tile_pool(name="obp", bufs=3))

            def oproj(sc):
                for n in range(4):
                    po = pso.tile([P, 512], f32, tag="o", name=f"po{sc}_{n}")
                    for t in range(4):
                        nc.tensor.matmul(
                            po[:], attnT[:, t, sc * P:(sc + 1) * P],
                            woT[:, t, n * 512:(n + 1) * 512],
                            start=(t == 0), stop=(t == 3))
                    ob = opool.tile([P, 512], f32, tag="ob", name=f"ob{sc}_{n}")
                    if n % 2 == 0:
                        nc.scalar.copy(ob[:], po[:])
                    else:
                        nc.vector.tensor_copy(out=ob[:], in_=po[:])
                    nc.sync.dma_start(
                        out_d[sc * P:(sc + 1) * P, n * 512:(n + 1) * 512], ob[:])

            for i in range(NSC):
                L = (i + 1) * P
                Lp = L if L % 256 == 0 else L + P
                halves = [(0, min(Lp, 1024))]
                if Lp > 1024:
                    halves.append((1024, Lp - 1024))
                for h in range(2):
                    mx = dpool.tile([P, 2], f32, tag="mx")
                    pss_tiles = []
                    for hf, (off, ln) in enumerate(halves):
                        ps = pss.tile([P, 1024], f32, tag="s",
                                      name=f"ps{i}_{h}_{hf}")
                        pss_tiles.append(ps)
                        for c in range(0, ln, 512):
                            w = min(512, ln - c)
                            for dc in range(2):
                                nc.tensor.matmul(
                                    ps[:, c:c + w],
                                    qT[:, h, dc, i * P:(i + 1) * P],
                                    kT[:, dc, off + c:off + c + w],
                                    start=(dc == 0), stop=(dc == 1))
                        if i * P >= off and i * P < off + ln:
                            db = i * P - off   # diag block col within half
                            nc.vector.tensor_add(ps[:, db:db + P],
                                                 ps[:, db:db + P], mdiag[:])
                        ln_real = min(L - off, ln)
                        nc.vector.tensor_reduce(
                            out=mx[:, hf:hf + 1], in_=ps[:, 0:ln_real],
                            axis=mybir.AxisListType.X, op=mybir.AluOpType.max)
                    mxf = dpool.tile([P, 1], f32, tag="mxf")
                    if len(halves) > 1:
                        nc.vector.tensor_tensor(out=mxf[:], in0=mx[:, 0:1],
                                                in1=mx[:, 1:2],
                                                op=mybir.AluOpType.max)
                    else:
                        nc.vector.tensor_copy(out=mxf[:], in_=mx[:, 0:1])
                    rq = rq_all[:, i, h:h + 1]
                    negmax = dpool.tile([P, 1], f32, tag="ngm")
                    nc.vector.tensor_scalar(out=negmax[:], in0=mxf[:],
                                            scalar1=rq, scalar2=-1.0,
                                            op0=mybir.AluOpType.mult,
                                            op1=mybir.AluOpType.mult)
                    pr = ppool.tile([P, 2048], f32, tag="pr")
                    den = dpool.tile([P, 2], f32, tag="den")
                    for hf, (off, ln) in enumerate(halves):
                        ln_real = min(L - off, ln)
                        nc.scalar.activation(pr[:, off:off + ln_real],
                                             pss_tiles[hf][:, 0:ln_real],
                                             ACT.Exp, bias=negmax[:], scale=rq,
                                             accum_out=den[:, hf:hf + 1])
                    denf = dpool.tile([P, 1], f32, tag="denf")
                    if len(halves) > 1:
                        nc.vector.tensor_add(denf[:], den[:, 0:1], den[:, 1:2])
                    else:
                        nc.vector.tensor_copy(out=denf[:], in_=den[:, 0:1])
                    rden = dpool.tile([P, 1], f32, tag="rden")
                    nc.vector.reciprocal(rden[:], denf[:])

                    pv = psv.tile([P, HD], f32, tag="pv")
                    for p0 in range(0, i + 1, 2):
                        cnt = min(2, i + 1 - p0)
                        pt = pst.tile([P, 2 * P], f32, tag="t")
                        for z in range(cnt):
                            nc.tensor.transpose(
                                pt[:, z * P:(z + 1) * P],
                                pr[:, (p0 + z) * P:(p0 + z + 1) * P], ident[:])
                        pts = tpool.tile([P, 2 * P], mdt, tag="pts")
                        if (p0 // 2) % 2 == 0:
                            nc.scalar.copy(pts[:, 0:cnt * P], pt[:, 0:cnt * P])
                        else:
                            nc.vector.tensor_copy(out=pts[:, 0:cnt * P],
                                                  in_=pt[:, 0:cnt * P])
                        for z in range(cnt):
                            kb = p0 + z
                            nc.tensor.matmul(pv[:], pts[:, z * P:(z + 1) * P],
                                             v_sb[:, kb, :],
                                             start=(kb == 0), stop=(kb == i))
                    attn_s = apool.tile([P, HD], f32, tag="attn")
                    nc.scalar.copy(attn_s[:], pv[:])
                    nc.vector.tensor_scalar_mul(out=attn_s[:], in0=attn_s[:],
                                                scalar1=rden[:])
                    pt = pst.tile([P, 2 * P], f32, tag="t")
                    for dc in range(2):
                        nc.tensor.transpose(pt[:, dc * P:(dc + 1) * P],
                                            attn_s[:, dc * P:(dc + 1) * P],
                                            ident[:])
                    dst = attnT[:, h * 2:h * 2 + 2, i * P:(i + 1) * P]
                    if h == 0:
                        nc.scalar.copy(dst, pt[:].rearrange(
                            "p (a b) -> p a b", a=2))
                    else:
                        nc.vector.tensor_copy(out=dst, in_=pt[:].rearrange(
                            "p (a b) -> p a b", a=2))
                if i >= 1:
                    oproj(i - 1)
            oproj(NSC - 1)

    nc.compile()
    return nc


def prep_core_inputs(inputs, core, use_f32r=True):
    """Host-side sharding for one core. Returns the in_map dict."""
    cvt = to_f32r if use_f32r else (lambda a: np.asarray(a, np.float32))
    b, kv, qp = core // 4, (core % 4) // 2, core % 2
    hq0 = kv * 4 + qp * 2           # first of the two query heads
    hidden = np.asarray(inputs["hidden_states"], np.float32)
    cos = np.asarray(inputs["cos"], np.float32)
    sin = np.asarray(inputs["sin"], np.float32)
    Wq = np.asarray(inputs["Wq"], np.float32)
    Wk = np.asarray(inputs["Wk"], np.float32)
    Wv = np.asarray(inputs["Wv"], np.float32)
    Wo = np.asarray(inputs["Wo"], np.float32)
    qw = np.asarray(inputs["q_norm_w"], np.float32)
    kw = np.asarray(inputs["k_norm_w"], np.float32)

    hT = np.ascontiguousarray(hidden[b].T).reshape(NHC, P, S)
    Wq_c = Wq[hq0 * HD:(hq0 + 2) * HD]          # [512, HID]
    Wk_c = Wk[kv * HD:(kv + 1) * HD]            # [256, HID]
    Wv_c = Wv[kv * HD:(kv + 1) * HD]
    wT = np.ascontiguousarray(
        np.concatenate([Wq_c.T, Wk_c.T, Wv_c.T], axis=1)).reshape(NHC, P, 1024)

    def cs_pack(w, cb, sb):
        rot_w = np.concatenate([w[P:], w[:P]])   # w[(d+128)%256]
        cosw = cb * w[None, :]
        sinw = sb * rot_w[None, :]
        sinw[:, :P] *= -1.0
        return np.ascontiguousarray(
            np.concatenate([cosw, sinw], axis=1)).reshape(NSC, P, 2 * HD)

    csq = cs_pack(qw, cos[b], sin[b])
    csk = cs_pack(kw, cos[b], sin[b])
    woT = np.ascontiguousarray(
        Wo[:, hq0 * HD:(hq0 + 2) * HD].T).reshape(4, P, HID)
    return {"hT": cvt(hT), "wT": cvt(wT),
            "csq": csq.astype(np.float32), "csk": csk.astype(np.float32),
            "woT": cvt(woT)}


def mask_is_causal(mask):
    m = np.asarray(mask)
    tri = np.tril(np.ones((S, S), dtype=bool))
    for b in range(m.shape[0]):
        mb = m[b, 0]
        if not (mb[tri] == 0.0).all():
            return False
        if not (mb[~tri] <= -1e8).all():
            return False
    return True


def reference_numpy(inputs, f64=True):
    """Defensive fallback for non-causal masks (never hit in practice)."""
    dt = np.float64 if f64 else np.float32
    hs = np.asarray(inputs["hidden_states"], dt)
    cos = np.asarray(inputs["cos"], dt)
    sin = np.asarray(inputs["sin"], dt)
    mask = np.asarray(inputs["attention_mask"], dt)
    Wq, Wk, Wv, Wo = (np.asarray(inputs[k], dt)
                      for k in ("Wq", "Wk", "Wv", "Wo"))
    qw = np.asarray(inputs["q_norm_w"], dt)
    kw = np.asarray(inputs["k_norm_w"], dt)

    def rms(x, w):
        return x / np.sqrt((x * x).mean(-1, keepdims=True) + EPS) * w

    def rope(x, c, s):
        x1, x2 = x[..., :HD // 2], x[..., HD // 2:]
        rot = np.concatenate([-x2, x1], axis=-1)
        return x * c[:, :, None, :] + rot * s[:, :, None, :]

    b, s_, _ = hs.shape
    q = (hs @ Wq.T).reshape(b, s_, NH, HD)
    k = (hs @ Wk.T).reshape(b, s_, NKV, HD)
    v = (hs @ Wv.T).reshape(b, s_, NKV, HD)
    q = rope(rms(q, qw), cos, sin).transpose(0, 2, 1, 3)
    k = rope(rms(k, kw), cos, sin).transpose(0, 2, 1, 3)
    v = rms(v, 1.0).transpose(0, 2, 1, 3)
    k = np.repeat(k, NH // NKV, axis=1)
    v = np.repeat(v, NH // NKV, axis=1)
    sc = np.einsum("bhqd,bhkd->bhqk", q, k) + mask
    sc = sc - sc.max(-1, keepdims=True)
    p = np.exp(sc)
    p /= p.sum(-1, keepdims=True)
    o = np.einsum("bhqk,bhkd->bqhd", p, v).reshape(b, s_, NH * HD)
    return (o @ Wo.T).astype(np.float32)


_PROGRAM = {}


def get_program(use_f32r=True, use_tmr=False):
    key = (use_f32r, use_tmr)
    if key not in _PROGRAM:
        _PROGRAM[key] = build_program(use_f32r=use_f32r, use_tmr=use_tmr)
    return _PROGRAM[key]


def run_on_hw(inputs, use_f32r=True, use_tmr=False, trace=False, **kw):
    from concourse.bass_utils import run_bass_kernel_spmd

    nc = get_program(use_f32r=use_f32r, use_tmr=use_tmr)
    in_maps = [prep_core_inputs(inputs, c, use_f32r) for c in range(8)]
    br = run_bass_kernel_spmd(nc, in_maps, list(range(8)), trace=trace, **kw)
    out = np.empty((B, S, HID), np.float32)
    for b in range(B):
        out[b] = br.results[4 * b]["out"] + br.results[4 * b + 1]["out"] \
            + br.results[4 * b + 2]["out"] + br.results[4 * b + 3]["out"]
    return out, br


def kernel(**inputs):
    if not mask_is_causal(inputs["attention_mask"]):
        return reference_numpy(inputs)
    out, _ = run_on_hw(inputs, use_f32r=True, trace=False)
    return out



# revision 19
# speedup vs baseline: 1.5099x; 1.5099x over previous
"""Gemma3n text attention on 8 Trainium2 NeuronCores (Bass/Tile).

Sharding: core c = b*4 + kv*2 + qp handles batch b, KV head kv and the
q-head pair (kv*4 + qp*2, kv*4 + qp*2 + 1).  Each core computes the
Q/K/V projections for its shard, QK-norm + RoPE, causal attention for
its two query heads, and a partial output projection against its
512-column slice of Wo.  The host sums the four partials per batch.

Phase B uses a transposed softmax: scores are computed directly as
scT[k, q] (kT as stationary operand), the softmax shift is a global
constant (exact softmax is shift-invariant; the causal diagonal keeps
every row's max - shift inside fp32 exp range), the denominator comes
from a ones[128,128]-stationary matmul that lands the per-column sum
broadcast across all partitions, and P@V directly produces the
transposed attention output attnT[d, q] that the output projection
wants.  No PE transposes of the probabilities are needed at all.

Self-contained: only needs numpy + the concourse tree that ships in the
container image (on PYTHONPATH at /root/.axon_site/_ro/trn_rl_repo).
"""

import sys

for _p in ("/root/.axon_site/_ro/trn_rl_repo", "/opt/trn_rl_repo"):
    if _p not in sys.path:
        sys.path.append(_p)

from contextlib import ExitStack

import numpy as np

import concourse.bass as bass
import concourse.mybir as mybir
import concourse.tile as tile
from concourse import bacc
from concourse.masks import make_identity

P = 128
B, S, HID = 2, 2048, 2048
NH, NKV, HD = 8, 2, 256
DQ = 2 * HD            # q-width per core (2 heads)
NSC = S // P           # 16 seq chunks
NHC = HID // P         # 16 hidden chunks
EPS = 1e-6
SHIFT = 35.0           # global softmax shift; see note above

f32 = mybir.dt.float32
f32r = mybir.dt.float32r
bf16 = mybir.dt.bfloat16
ACT = mybir.ActivationFunctionType


def to_f32r(arr):
    """Round fp32 -> fp32r bit format (11 explicit mantissa bits, RNE).

    Bit-exact with libwalrus fp32_to_fp32r."""
    u = np.ascontiguousarray(arr, np.float32).view(np.uint32)
    r = ((u.astype(np.uint64) + 0x7FF + ((u >> 12) & 1)) & 0xFFFFF000)
    return r.astype(np.uint32).view(np.float32)


def build_program(use_f32r=True, debug_dump=False):
    """Emit the SPMD per-core program. Returns the compiled Bacc object."""
    nc = bacc.Bacc("TRN2", target_bir_lowering=False, debug=False, num_devices=8)

    mdt = f32r if use_f32r else f32   # dtype of fp32-precision matmul operands

    hT_d = nc.dram_tensor("hT", [NHC, P, S], mdt, kind="ExternalInput")
    wT_d = nc.dram_tensor("wT", [NHC, P, DQ + 2 * HD], mdt, kind="ExternalInput")
    csq_d = nc.dram_tensor("csq", [NSC, P, 2 * HD], f32, kind="ExternalInput")
    csk_d = nc.dram_tensor("csk", [NSC, P, 2 * HD], f32, kind="ExternalInput")
    woT_d = nc.dram_tensor("woT", [4, P, HID], bf16, kind="ExternalInput")
    out_d = nc.dram_tensor("out", [S, HID], f32, kind="ExternalOutput")
    if debug_dump:
        qT_dbg = nc.dram_tensor("qT_dbg", [P, 2, 2, S], mdt, kind="ExternalOutput")
        kT_dbg = nc.dram_tensor("kT_dbg", [P, 2, S], mdt, kind="ExternalOutput")
        v_dbg = nc.dram_tensor("v_dbg", [P, NSC, HD], bf16, kind="ExternalOutput")
        at_dbg = nc.dram_tensor("at_dbg", [P, 4, S], bf16, kind="ExternalOutput")
        rd_dbg = nc.dram_tensor("rd_dbg", [16, P, 256], f32, kind="ExternalOutput")

    with tile.TileContext(nc) as tc, ExitStack() as ctx:
        const = ctx.enter_context(tc.tile_pool(name="const", bufs=1))
        persist = ctx.enter_context(tc.tile_pool(name="persist", bufs=1))

        ident = const.tile([P, P], f32)
        make_identity(nc, ident)
        # mdiagT[k, q] = 0 where k <= q (valid), -1e9 above (k > q)
        mdiagT = const.tile([P, P], f32)
        nc.gpsimd.memset(mdiagT, 0.0)
        nc.gpsimd.affine_select(out=mdiagT, in_=mdiagT,
                                compare_op=mybir.AluOpType.is_ge, fill=-1e9,
                                base=0, pattern=[[1, P]], channel_multiplier=-1)
        ones_k = const.tile([P, P], bf16)
        nc.vector.memset(ones_k, 1.0)
        eps_t = const.tile([P, 1], f32)
        nc.vector.memset(eps_t, EPS)
        nshift = const.tile([P, 1], f32)
        nc.vector.memset(nshift, -SHIFT)

        # persistent SBUF tensors
        qT = persist.tile([P, 2, 2, S], mdt)      # [d, head, dchunk, qpos]
        kT = persist.tile([P, 2, S], mdt)         # [d, dchunk, kpos]
        v_sb = persist.tile([P, NSC, HD], bf16)   # [kpos, kchunk, d]

        # Wo slice, loaded early so the DMA hides under phase A
        wopool = ctx.enter_context(tc.tile_pool(name="wo", bufs=1))
        woT = wopool.tile([P, 4, HID], bf16)
        for t in range(4):
            nc.sync.dma_start(woT[:, t, :], woT_d[t])

        # ------- Phase A: QKV proj + norm + rope + transposes (fused) --------
        with ExitStack() as a1:
            hpool = a1.enter_context(tc.tile_pool(name="hTp", bufs=20))
            wpool = a1.enter_context(tc.tile_pool(name="wTp", bufs=1))
            wt_all = wpool.tile([P, NHC, DQ + 2 * HD], mdt)
            nc.sync.dma_start(wt_all, wT_d.ap().rearrange("h p d -> p h d"))
            cpool = a1.enter_context(tc.tile_pool(name="cs", bufs=3))
            epool = a1.enter_context(tc.tile_pool(name="evict", bufs=4))
            spool = a1.enter_context(tc.tile_pool(name="small", bufs=8))
            psA = a1.enter_context(tc.tile_pool(name="psA", bufs=6, space="PSUM"))
            psT = a1.enter_context(tc.tile_pool(name="psT", bufs=2, space="PSUM"))

            def tail_psum(sc, psq_j, pskv_j, csq, csk, qro, kro):
                """norm + rope for one seq chunk (scalar/DVE, reads PSUM)."""
                # sum of squares per 256-group via ACT Square (reads PSUM)
                ssq = spool.tile([P, 4], f32, tag="ssq", name=f"ssq{sc}")
                scr = epool.tile([P, HD], f32, tag="scr", name=f"scr{sc}")
                nc.scalar.activation(scr[:], psq_j[:, 0:HD], ACT.Square,
                                     accum_out=ssq[:, 0:1])
                nc.scalar.activation(scr[:], psq_j[:, HD:2 * HD],
                                     ACT.Square, accum_out=ssq[:, 1:2])
                nc.scalar.activation(scr[:], pskv_j[:, 0:HD], ACT.Square,
                                     accum_out=ssq[:, 2:3])
                nc.scalar.activation(scr[:], pskv_j[:, HD:2 * HD],
                                     ACT.Square, accum_out=ssq[:, 3:4])
                rstd = spool.tile([P, 4], f32, tag="rstd", name=f"rstd{sc}")
                nc.scalar.activation(rstd[:], ssq[:], ACT.Sqrt,
                                     bias=eps_t[:], scale=1.0 / HD)
                nc.vector.reciprocal(rstd[:], rstd[:])

                # v: scale + evict in one DVE op
                nc.vector.tensor_scalar_mul(out=v_sb[:, sc, :],
                                            in0=pskv_j[:, HD:2 * HD],
                                            scalar1=rstd[:, 3:4])

                # rope(x) = x*cosw + swap(x)*sinw (sinw lo pre-negated);
                # reads projection PSUM directly, writes SBUF
                for h in range(2):
                    b0 = h * HD
                    tmp = epool.tile([P, HD], f32, tag="tmp", name=f"tq{sc}_{h}")
                    nc.vector.tensor_mul(tmp[:, 0:P],
                                         psq_j[:, b0 + P:b0 + HD],
                                         csq[:, HD:HD + P])
                    nc.vector.tensor_mul(tmp[:, P:HD],
                                         psq_j[:, b0:b0 + P],
                                         csq[:, HD + P:2 * HD])
                    qh = qro[:, b0:b0 + HD]
                    nc.vector.tensor_mul(qh, psq_j[:, b0:b0 + HD],
                                         csq[:, 0:HD])
                    nc.vector.tensor_add(qh, qh, tmp[:])
                    nc.vector.tensor_scalar_mul(out=qh, in0=qh,
                                                scalar1=rstd[:, h:h + 1])
                tmp = epool.tile([P, HD], f32, tag="tmp", name=f"tk{sc}")
                nc.vector.tensor_mul(tmp[:, 0:P], pskv_j[:, P:HD],
                                     csk[:, HD:HD + P])
                nc.vector.tensor_mul(tmp[:, P:HD], pskv_j[:, 0:P],
                                     csk[:, HD + P:2 * HD])
                nc.vector.tensor_mul(kro[:], pskv_j[:, 0:HD], csk[:, 0:HD])
                nc.vector.tensor_add(kro[:], kro[:], tmp[:])
                nc.vector.tensor_scalar_mul(out=kro[:], in0=kro[:],
                                            scalar1=rstd[:, 2:3])

            def tail_pe(sc, qro, kro):
                """transposes into qT/kT (PE) + paired evictions."""
                for h in range(2):
                    pt = psT.tile([P, 2 * P], f32, tag="t", name=f"ptq{sc}_{h}")
                    for dc in range(2):
                        nc.tensor.transpose(
                            pt[:, dc * P:(dc + 1) * P],
                            qro[:, h * HD + dc * P:h * HD + (dc + 1) * P],
                            ident[:])
                    dst = qT[:, h, 0:2, sc * P:(sc + 1) * P]
                    if (sc + h) % 2 == 0:
                        nc.scalar.copy(dst, pt[:].rearrange(
                            "p (a b) -> p a b", a=2))
                    else:
                        nc.vector.tensor_copy(out=dst, in_=pt[:].rearrange(
                            "p (a b) -> p a b", a=2))
                pt = psT.tile([P, 2 * P], f32, tag="t", name=f"ptk{sc}")
                for dc in range(2):
                    nc.tensor.transpose(pt[:, dc * P:(dc + 1) * P],
                                        kro[:, dc * P:(dc + 1) * P],
                                        ident[:])
                dst = kT[:, 0:2, sc * P:(sc + 1) * P]
                if sc % 2 == 0:
                    nc.vector.tensor_copy(out=dst, in_=pt[:].rearrange(
                        "p (a b) -> p a b", a=2))
                else:
                    nc.scalar.copy(dst, pt[:].rearrange(
                        "p (a b) -> p a b", a=2))

            # one group per seq chunk (2 PSUM tiles each; bufs=6 gives the
            # delayed tails three groups of slack before bank recycling)
            ths = None
            prev = None          # (sc, psq, pskv, csq, csk, qro, kro)
            for g in range(NSC):
                if g % 2 == 0:   # th tiles span two groups (1KB DMA lines)
                    ths = []
                    for hc in range(NHC):
                        th = hpool.tile([P, 2 * P], mdt, tag="h",
                                        name=f"th{g}_{hc}")
                        nc.sync.dma_start(th, hT_d[hc, :, g * P:(g + 2) * P])
                        ths.append(th)
                csq = cpool.tile([P, 2 * HD], f32, tag="csq", name=f"csq{g}")
                nc.sync.dma_start(csq, csq_d[g])
                csk = cpool.tile([P, 2 * HD], f32, tag="csk", name=f"csk{g}")
                nc.sync.dma_start(csk, csk_d[g])
                psq = psA.tile([P, DQ], f32, tag="ps", name=f"psq{g}")
                pskv = psA.tile([P, 2 * HD], f32, tag="ps", name=f"pskv{g}")
                qro = epool.tile([P, DQ], f32, tag="qro", name=f"qro{g}")
                kro = epool.tile([P, HD], f32, tag="kro", name=f"kro{g}")
                for hc in range(NHC):
                    lhs = ths[hc][:, (g % 2) * P:(g % 2 + 1) * P]
                    tw = wt_all[:, hc]
                    st, sp = hc == 0, hc == NHC - 1
                    nc.tensor.matmul(psq[:], lhs, tw[:, 0:DQ],
                                     start=st, stop=sp)
                    nc.tensor.matmul(pskv[:], lhs, tw[:, DQ:],
                                     start=st, stop=sp)
                    if prev is not None:
                        if hc == 2:
                            tail_psum(*prev)
                        elif hc == 12:
                            tail_pe(prev[0], prev[5], prev[6])
                prev = (g, psq, pskv, csq, csk, qro, kro)
            tail_psum(*prev)
            tail_pe(prev[0], prev[5], prev[6])

        if debug_dump:
            nc.sync.dma_start(qT_dbg.ap(), qT[:])
            nc.sync.dma_start(kT_dbg.ap(), kT[:])
            nc.sync.dma_start(v_dbg.ap(), v_sb[:])

        # ---------------- Phase B: transposed-softmax attention --------------
        with ExitStack() as bctx:
            # NB: every matmul start=True clears its whole PSUM bank, so each
            # concurrently-accumulating group needs its own bank/pool.
            psS = bctx.enter_context(tc.tile_pool(name="psS", bufs=3, space="PSUM"))
            psV0 = bctx.enter_context(tc.tile_pool(name="psV0", bufs=1, space="PSUM"))
            psV1 = bctx.enter_context(tc.tile_pool(name="psV1", bufs=1, space="PSUM"))
            psD = bctx.enter_context(tc.tile_pool(name="psD", bufs=1, space="PSUM"))
            psO = bctx.enter_context(tc.tile_pool(name="psO", bufs=2, space="PSUM"))
            expool = bctx.enter_context(tc.tile_pool(name="expp", bufs=4))
            rpool = bctx.enter_context(tc.tile_pool(name="rdp", bufs=2))
            opool = bctx.enter_context(tc.tile_pool(name="obp", bufs=2))
            apool = bctx.enter_context(tc.tile_pool(name="attp", bufs=1))
            attnT = apool.tile([P, 4, S], bf16)   # [d2, (h,dc), qpos]

            def oproj(sc):
                ob = opool.tile([P, HID], f32, tag="ob", name=f"ob{sc}")
                for n in range(4):
                    po = psO.tile([P, 512], f32, tag="po", name=f"po{sc}_{n}")
                    for t in range(4):
                        nc.tensor.matmul(
                            po[:], attnT[:, t, sc * P:(sc + 1) * P],
                            woT[:, t, n * 512:(n + 1) * 512],
                            start=(t == 0), stop=(t == 3))
                    if n % 2 == 0:
                        nc.scalar.copy(ob[:, n * 512:(n + 1) * 512], po[:])
                    else:
                        nc.vector.tensor_copy(out=ob[:, n * 512:(n + 1) * 512],
                                              in_=po[:])
                nc.sync.dma_start(out_d[sc * P:(sc + 1) * P, :], ob[:])

            pend_oproj = []
            for p in range(8):
                K = 2 * p + 2
                q0 = p * 256
                # diagonal (masked) blocks first: their extra DVE hop gets
                # hidden under the remaining kb's score matmuls
                order = [2 * p, 2 * p + 1] + list(range(2 * p))
                for h in range(2):
                    exp_tiles = {}
                    pv0 = psV0.tile([P, 256], f32, tag="pv0", name=f"pv0_{p}_{h}")
                    pv1 = psV1.tile([P, 256], f32, tag="pv1", name=f"pv1_{p}_{h}")
                    dnb = psD.tile([P, 256], f32, tag="dn", name=f"dn{p}_{h}")

                    def emit_sc(kb, p=p, h=h, q0=q0, exp_tiles=exp_tiles):
                        t = psS.tile([P, 256], f32, tag="sc",
                                     name=f"sc{p}_{h}_{kb}")
                        for dc in range(2):
                            nc.tensor.matmul(
                                t[:], kT[:, dc, kb * P:(kb + 1) * P],
                                qT[:, h, dc, q0:q0 + 256],
                                start=(dc == 0), stop=(dc == 1))
                        if kb == 2 * p:
                            nc.vector.tensor_add(t[:, 0:P], t[:, 0:P],
                                                 mdiagT[:])
                        elif kb == 2 * p + 1:
                            nc.vector.memset(t[:, 0:P], -1e9)
                            nc.vector.tensor_add(t[:, P:2 * P], t[:, P:2 * P],
                                                 mdiagT[:])
                        e = expool.tile([P, 256], bf16, tag="exp",
                                        name=f"ex{p}_{h}_{kb}")
                        nc.scalar.activation(e[:], t[:], ACT.Exp,
                                             bias=nshift[:])
                        exp_tiles[kb] = e

                    emit_sc(order[0])
                    # a deferred output projection fills the mask+exp latency
                    if pend_oproj:
                        oproj(pend_oproj.pop(0))
                    emit_sc(order[1])
                    for i, kb in enumerate(order):
                        e = exp_tiles.pop(kb)
                        st, sp = i == 0, i == K - 1
                        nc.tensor.matmul(pv0[:], v_sb[:, kb, 0:P],
                                         e[:], start=st, stop=sp)
                        nc.tensor.matmul(pv1[:], v_sb[:, kb, P:HD],
                                         e[:], start=st, stop=sp)
                        nc.tensor.matmul(dnb[:], ones_k[:, 0:P], e[:],
                                         start=st, stop=sp)
                        if i + 2 < K:
                            emit_sc(order[i + 2])
                    rd = rpool.tile([P, 256], f32, tag="rd", name=f"rd{p}_{h}")
                    nc.vector.reciprocal(rd[:], dnb[:])
                    if debug_dump:
                        nc.sync.dma_start(rd_dbg[p * 2 + h], rd[:])
                    nc.vector.tensor_mul(attnT[:, h * 2, q0:q0 + 256],
                                         pv0[:], rd[:])
                    nc.vector.tensor_mul(attnT[:, h * 2 + 1, q0:q0 + 256],
                                         pv1[:], rd[:])
                pend_oproj += [2 * p, 2 * p + 1]
            for sc in pend_oproj:
                oproj(sc)
            if debug_dump:
                nc.sync.dma_start(at_dbg.ap(), attnT[:])

    nc.compile()
    return nc


def prep_core_inputs(inputs, core, use_f32r=True):
    """Host-side sharding for one core. Returns the in_map dict."""
    cvt = to_f32r if use_f32r else (lambda a: np.asarray(a, np.float32))
    bf16np = mybir.dt.np(bf16)
    b, kv, qp = core // 4, (core % 4) // 2, core % 2
    hq0 = kv * 4 + qp * 2           # first of the two query heads
    hidden = np.asarray(inputs["hidden_states"], np.float32)
    cos = np.asarray(inputs["cos"], np.float32)
    sin = np.asarray(inputs["sin"], np.float32)
    Wq = np.asarray(inputs["Wq"], np.float32)
    Wk = np.asarray(inputs["Wk"], np.float32)
    Wv = np.asarray(inputs["Wv"], np.float32)
    Wo = np.asarray(inputs["Wo"], np.float32)
    qw = np.asarray(inputs["q_norm_w"], np.float32)
    kw = np.asarray(inputs["k_norm_w"], np.float32)

    hT = np.ascontiguousarray(hidden[b].T).reshape(NHC, P, S)
    Wq_c = Wq[hq0 * HD:(hq0 + 2) * HD]          # [512, HID]
    Wk_c = Wk[kv * HD:(kv + 1) * HD]            # [256, HID]
    Wv_c = Wv[kv * HD:(kv + 1) * HD]
    wT = np.ascontiguousarray(
        np.concatenate([Wq_c.T, Wk_c.T, Wv_c.T], axis=1)).reshape(NHC, P, 1024)

    def cs_pack(w, cb, sb):
        rot_w = np.concatenate([w[P:], w[:P]])   # w[(d+128)%256]
        cosw = cb * w[None, :]
        sinw = sb * rot_w[None, :]
        sinw[:, :P] *= -1.0
        return np.ascontiguousarray(
            np.concatenate([cosw, sinw], axis=1)).reshape(NSC, P, 2 * HD)

    csq = cs_pack(qw, cos[b], sin[b])
    csk = cs_pack(kw, cos[b], sin[b])
    woT = np.ascontiguousarray(
        Wo[:, hq0 * HD:(hq0 + 2) * HD].T).reshape(4, P, HID)
    return {"hT": cvt(hT), "wT": cvt(wT),
            "csq": csq.astype(np.float32), "csk": csk.astype(np.float32),
            "woT": woT.astype(bf16np)}


def mask_is_causal(mask):
    m = np.asarray(mask)
    tri = np.tril(np.ones((S, S), dtype=bool))
    for b in range(m.shape[0]):
        mb = m[b, 0]
        if not (mb[tri] == 0.0).all():
            return False
        if not (mb[~tri] <= -1e8).all():
            return False
    return True


def reference_numpy(inputs, f64=True):
    """Defensive fallback for non-causal masks (never hit in practice)."""
    dt = np.float64 if f64 else np.float32
    hs = np.asarray(inputs["hidden_states"], dt)
    cos = np.asarray(inputs["cos"], dt)
    sin = np.asarray(inputs["sin"], dt)
    mask = np.asarray(inputs["attention_mask"], dt)
    Wq, Wk, Wv, Wo = (np.asarray(inputs[k], dt)
                      for k in ("Wq", "Wk", "Wv", "Wo"))
    qw = np.asarray(inputs["q_norm_w"], dt)
    kw = np.asarray(inputs["k_norm_w"], dt)

    def rms(x, w):
        return x / np.sqrt((x * x).mean(-1, keepdims=True) + EPS) * w

    def rope(x, c, s):
        x1, x2 = x[..., :HD // 2], x[..., HD // 2:]
        rot = np.concatenate([-x2, x1], axis=-1)
        return x * c[:, :, None, :] + rot * s[:, :, None, :]

    b, s_, _ = hs.shape
    q = (hs @ Wq.T).reshape(b, s_, NH, HD)
    k = (hs @ Wk.T).reshape(b, s_, NKV, HD)
    v = (hs @ Wv.T).reshape(b, s_, NKV, HD)
    q = rope(rms(q, qw), cos, sin).transpose(0, 2, 1, 3)
    k = rope(rms(k, kw), cos, sin).transpose(0, 2, 1, 3)
    v = rms(v, 1.0).transpose(0, 2, 1, 3)
    k = np.repeat(k, NH // NKV, axis=1)
    v = np.repeat(v, NH // NKV, axis=1)
    sc = np.einsum("bhqd,bhkd->bhqk", q, k) + mask
    sc = sc - sc.max(-1, keepdims=True)
    p = np.exp(sc)
    p /= p.sum(-1, keepdims=True)
    o = np.einsum("bhqk,bhkd->bqhd", p, v).reshape(b, s_, NH * HD)
    return (o @ Wo.T).astype(np.float32)


_PROGRAM = {}


def get_program(use_f32r=True, debug_dump=False):
    key = (use_f32r, debug_dump)
    if key not in _PROGRAM:
        _PROGRAM[key] = build_program(use_f32r=use_f32r,
                                      debug_dump=debug_dump)
    return _PROGRAM[key]


def run_on_hw(inputs, use_f32r=True, trace=False, debug_dump=False, **kw):
    from concourse.bass_utils import run_bass_kernel_spmd

    nc = get_program(use_f32r=use_f32r, debug_dump=debug_dump)
    in_maps = [prep_core_inputs(inputs, c, use_f32r) for c in range(8)]
    br = run_bass_kernel_spmd(nc, in_maps, list(range(8)), trace=trace, **kw)
    out = np.empty((B, S, HID), np.float32)
    for b in range(B):
        out[b] = br.results[4 * b]["out"] + br.results[4 * b + 1]["out"] \
            + br.results[4 * b + 2]["out"] + br.results[4 * b + 3]["out"]
    return out, br


def kernel(**inputs):
    if not mask_is_causal(inputs["attention_mask"]):
        return reference_numpy(inputs)
    out, _ = run_on_hw(inputs, use_f32r=True, trace=False)
    return out


# revision 22
# speedup vs baseline: 1.5760x; 1.0437x over previous
"""Gemma3n text attention on 8 Trainium2 NeuronCores (Bass/Tile).

Sharding: core c = b*4 + kv*2 + qp handles batch b, KV head kv and the
q-head pair (kv*4 + qp*2, kv*4 + qp*2 + 1).  Each core computes the
Q/K/V projections for its shard, QK-norm + RoPE, causal attention for
its two query heads, and a partial output projection against its
512-column slice of Wo.  The host sums the four partials per batch.

Phase B uses a transposed softmax: scores are computed directly as
scT[k, q] (kT as stationary operand), the softmax shift is a global
constant (exact softmax is shift-invariant; the causal diagonal keeps
every row's max - shift inside fp32 exp range), the denominator comes
from a ones[128,128]-stationary matmul that lands the per-column sum
broadcast across all partitions, and P@V directly produces the
transposed attention output attnT[d, q] that the output projection
wants.  No PE transposes of the probabilities are needed at all.

Self-contained: only needs numpy + the concourse tree that ships in the
container image (on PYTHONPATH at /root/.axon_site/_ro/trn_rl_repo).
"""

import sys

for _p in ("/root/.axon_site/_ro/trn_rl_repo", "/opt/trn_rl_repo"):
    if _p not in sys.path:
        sys.path.append(_p)

from contextlib import ExitStack

import numpy as np

import concourse.bass as bass
import concourse.mybir as mybir
import concourse.tile as tile
from concourse import bacc
from concourse.masks import make_identity

P = 128
B, S, HID = 2, 2048, 2048
NH, NKV, HD = 8, 2, 256
DQ = 2 * HD            # q-width per core (2 heads)
NSC = S // P           # 16 seq chunks
NHC = HID // P         # 16 hidden chunks
EPS = 1e-6
SHIFT = 35.0           # global softmax shift; see note above

f32 = mybir.dt.float32
f32r = mybir.dt.float32r
bf16 = mybir.dt.bfloat16
ACT = mybir.ActivationFunctionType


def to_f32r(arr):
    """Round fp32 -> fp32r bit format (11 explicit mantissa bits, RNE).

    Bit-exact with libwalrus fp32_to_fp32r."""
    u = np.ascontiguousarray(arr, np.float32).view(np.uint32)
    r = ((u.astype(np.uint64) + 0x7FF + ((u >> 12) & 1)) & 0xFFFFF000)
    return r.astype(np.uint32).view(np.float32)


def build_program(use_f32r=True, debug_dump=False):
    """Emit the SPMD per-core program. Returns the compiled Bacc object."""
    nc = bacc.Bacc("TRN2", target_bir_lowering=False, debug=False, num_devices=8)

    mdt = f32r if use_f32r else f32   # dtype of fp32-precision matmul operands

    hT_d = nc.dram_tensor("hT", [NHC, P, S], mdt, kind="ExternalInput")
    wT_d = nc.dram_tensor("wT", [NHC, P, DQ + 2 * HD], mdt, kind="ExternalInput")
    csq_d = nc.dram_tensor("csq", [NSC, P, 2 * HD], f32, kind="ExternalInput")
    csk_d = nc.dram_tensor("csk", [NSC, P, 2 * HD], f32, kind="ExternalInput")
    woT_d = nc.dram_tensor("woT", [4, P, HID], bf16, kind="ExternalInput")
    out_d = nc.dram_tensor("out", [S, HID], f32, kind="ExternalOutput")
    if debug_dump:
        qT_dbg = nc.dram_tensor("qT_dbg", [P, 2, 2, S], mdt, kind="ExternalOutput")
        kT_dbg = nc.dram_tensor("kT_dbg", [P, 2, S], mdt, kind="ExternalOutput")
        v_dbg = nc.dram_tensor("v_dbg", [P, NSC, HD], bf16, kind="ExternalOutput")
        at_dbg = nc.dram_tensor("at_dbg", [P, 4, S], bf16, kind="ExternalOutput")
        rd_dbg = nc.dram_tensor("rd_dbg", [16, P, 256], f32, kind="ExternalOutput")

    with tile.TileContext(nc) as tc, ExitStack() as ctx:
        const = ctx.enter_context(tc.tile_pool(name="const", bufs=1))
        persist = ctx.enter_context(tc.tile_pool(name="persist", bufs=1))

        ident = const.tile([P, P], f32)
        make_identity(nc, ident)
        # mdiagT[k, q] = 0 where k <= q (valid), -1e9 above (k > q)
        mdiagT = const.tile([P, P], f32)
        nc.gpsimd.memset(mdiagT, 0.0)
        nc.gpsimd.affine_select(out=mdiagT, in_=mdiagT,
                                compare_op=mybir.AluOpType.is_ge, fill=-1e9,
                                base=0, pattern=[[1, P]], channel_multiplier=-1)
        ones_k = const.tile([P, P], bf16)
        nc.vector.memset(ones_k, 1.0)
        eps_t = const.tile([P, 1], f32)
        nc.vector.memset(eps_t, EPS)
        nshift = const.tile([P, 1], f32)
        nc.vector.memset(nshift, -SHIFT)

        # persistent SBUF tensors
        qT = persist.tile([P, 2, 2, S], mdt)      # [d, head, dchunk, qpos]
        kT = persist.tile([P, 2, S], mdt)         # [d, dchunk, kpos]
        v_sb = persist.tile([P, NSC, HD], bf16)   # [kpos, kchunk, d]

        # Wo slice pool (DMAs issued at the phase A/B boundary: phase A
        # compute hides them; issuing them first would delay the first
        # projection matmuls by ~10us of DMA queue time)
        wopool = ctx.enter_context(tc.tile_pool(name="wo", bufs=1))
        woT = wopool.tile([P, 4, HID], bf16)

        # ------- Phase A: QKV proj + norm + rope + transposes (fused) --------
        with ExitStack() as a1:
            hpool = a1.enter_context(tc.tile_pool(name="hTp", bufs=20))
            wpool = a1.enter_context(tc.tile_pool(name="wTp", bufs=1))
            wt_all = wpool.tile([P, NHC, DQ + 2 * HD], mdt)
            cpool = a1.enter_context(tc.tile_pool(name="cs", bufs=3))
            epool = a1.enter_context(tc.tile_pool(name="evict", bufs=4))
            spool = a1.enter_context(tc.tile_pool(name="small", bufs=8))
            psA = a1.enter_context(tc.tile_pool(name="psA", bufs=6, space="PSUM"))
            psT = a1.enter_context(tc.tile_pool(name="psT", bufs=2, space="PSUM"))

            def tail_psum(sc, psq_j, pskv_j, csq, csk, qro, kro):
                """norm + rope for one seq chunk (scalar/DVE, reads PSUM)."""
                # sum of squares per 256-group via ACT Square (reads PSUM)
                ssq = spool.tile([P, 4], f32, tag="ssq", name=f"ssq{sc}")
                scr = epool.tile([P, HD], f32, tag="scr", name=f"scr{sc}")
                nc.scalar.activation(scr[:], psq_j[:, 0:HD], ACT.Square,
                                     accum_out=ssq[:, 0:1])
                nc.scalar.activation(scr[:], psq_j[:, HD:2 * HD],
                                     ACT.Square, accum_out=ssq[:, 1:2])
                nc.scalar.activation(scr[:], pskv_j[:, 0:HD], ACT.Square,
                                     accum_out=ssq[:, 2:3])
                nc.scalar.activation(scr[:], pskv_j[:, HD:2 * HD],
                                     ACT.Square, accum_out=ssq[:, 3:4])
                rstd = spool.tile([P, 4], f32, tag="rstd", name=f"rstd{sc}")
                nc.scalar.activation(rstd[:], ssq[:], ACT.Sqrt,
                                     bias=eps_t[:], scale=1.0 / HD)
                nc.vector.reciprocal(rstd[:], rstd[:])

                # v: scale + evict in one DVE op
                nc.vector.tensor_scalar_mul(out=v_sb[:, sc, :],
                                            in0=pskv_j[:, HD:2 * HD],
                                            scalar1=rstd[:, 3:4])

                # rope(x) = x*cosw + swap(x)*sinw (sinw lo pre-negated);
                # reads projection PSUM directly, writes SBUF
                for h in range(2):
                    b0 = h * HD
                    tmp = epool.tile([P, HD], f32, tag="tmp", name=f"tq{sc}_{h}")
                    nc.vector.tensor_mul(tmp[:, 0:P],
                                         psq_j[:, b0 + P:b0 + HD],
                                         csq[:, HD:HD + P])
                    nc.vector.tensor_mul(tmp[:, P:HD],
                                         psq_j[:, b0:b0 + P],
                                         csq[:, HD + P:2 * HD])
                    qh = qro[:, b0:b0 + HD]
                    nc.vector.tensor_mul(qh, psq_j[:, b0:b0 + HD],
                                         csq[:, 0:HD])
                    nc.vector.tensor_add(qh, qh, tmp[:])
                    nc.vector.tensor_scalar_mul(out=qh, in0=qh,
                                                scalar1=rstd[:, h:h + 1])
                tmp = epool.tile([P, HD], f32, tag="tmp", name=f"tk{sc}")
                nc.vector.tensor_mul(tmp[:, 0:P], pskv_j[:, P:HD],
                                     csk[:, HD:HD + P])
                nc.vector.tensor_mul(tmp[:, P:HD], pskv_j[:, 0:P],
                                     csk[:, HD + P:2 * HD])
                nc.vector.tensor_mul(kro[:], pskv_j[:, 0:HD], csk[:, 0:HD])
                nc.vector.tensor_add(kro[:], kro[:], tmp[:])
                nc.vector.tensor_scalar_mul(out=kro[:], in0=kro[:],
                                            scalar1=rstd[:, 2:3])

            def tail_pe(sc, qro, kro):
                """transposes into qT/kT (PE) + paired evictions."""
                for h in range(2):
                    pt = psT.tile([P, 2 * P], f32, tag="t", name=f"ptq{sc}_{h}")
                    for dc in range(2):
                        nc.tensor.transpose(
                            pt[:, dc * P:(dc + 1) * P],
                            qro[:, h * HD + dc * P:h * HD + (dc + 1) * P],
                            ident[:])
                    dst = qT[:, h, 0:2, sc * P:(sc + 1) * P]
                    if (sc + h) % 2 == 0:
                        nc.scalar.copy(dst, pt[:].rearrange(
                            "p (a b) -> p a b", a=2))
                    else:
                        nc.vector.tensor_copy(out=dst, in_=pt[:].rearrange(
                            "p (a b) -> p a b", a=2))
                pt = psT.tile([P, 2 * P], f32, tag="t", name=f"ptk{sc}")
                for dc in range(2):
                    nc.tensor.transpose(pt[:, dc * P:(dc + 1) * P],
                                        kro[:, dc * P:(dc + 1) * P],
                                        ident[:])
                dst = kT[:, 0:2, sc * P:(sc + 1) * P]
                if sc % 2 == 0:
                    nc.vector.tensor_copy(out=dst, in_=pt[:].rearrange(
                        "p (a b) -> p a b", a=2))
                else:
                    nc.scalar.copy(dst, pt[:].rearrange(
                        "p (a b) -> p a b", a=2))

            # one group per seq chunk (2 PSUM tiles each; bufs=6 gives the
            # delayed tails three groups of slack before bank recycling)
            ths = None
            prev = None          # (sc, psq, pskv, csq, csk, qro, kro)
            for g in range(NSC):
                if g % 2 == 0:   # th tiles span two groups (1KB DMA lines)
                    ths = []
                    for hc in range(NHC):
                        if g == 0:
                            # interleave weight-slice and activation loads so
                            # the first matmul starts after one hc pair lands
                            nc.sync.dma_start(wt_all[:, hc, :], wT_d[hc])
                        th = hpool.tile([P, 2 * P], mdt, tag="h",
                                        name=f"th{g}_{hc}")
                        nc.sync.dma_start(th, hT_d[hc, :, g * P:(g + 2) * P])
                        ths.append(th)
                csq = cpool.tile([P, 2 * HD], f32, tag="csq", name=f"csq{g}")
                nc.sync.dma_start(csq, csq_d[g])
                csk = cpool.tile([P, 2 * HD], f32, tag="csk", name=f"csk{g}")
                nc.sync.dma_start(csk, csk_d[g])
                psq = psA.tile([P, DQ], f32, tag="ps", name=f"psq{g}")
                pskv = psA.tile([P, 2 * HD], f32, tag="ps", name=f"pskv{g}")
                qro = epool.tile([P, DQ], f32, tag="qro", name=f"qro{g}")
                kro = epool.tile([P, HD], f32, tag="kro", name=f"kro{g}")
                for hc in range(NHC):
                    lhs = ths[hc][:, (g % 2) * P:(g % 2 + 1) * P]
                    tw = wt_all[:, hc]
                    st, sp = hc == 0, hc == NHC - 1
                    nc.tensor.matmul(psq[:], lhs, tw[:, 0:DQ],
                                     start=st, stop=sp)
                    nc.tensor.matmul(pskv[:], lhs, tw[:, DQ:],
                                     start=st, stop=sp)
                    if prev is not None:
                        if hc == 2:
                            tail_psum(*prev)
                        elif hc == 12:
                            tail_pe(prev[0], prev[5], prev[6])
                prev = (g, psq, pskv, csq, csk, qro, kro)
                if g == 4:
                    # Wo load rides under the remaining phase A compute
                    for t in range(4):
                        nc.sync.dma_start(woT[:, t, :], woT_d[t])
            tail_psum(*prev)
            tail_pe(prev[0], prev[5], prev[6])

        if debug_dump:
            nc.sync.dma_start(qT_dbg.ap(), qT[:])
            nc.sync.dma_start(kT_dbg.ap(), kT[:])
            nc.sync.dma_start(v_dbg.ap(), v_sb[:])

        # ---------------- Phase B: transposed-softmax attention --------------
        with ExitStack() as bctx:
            # NB: every matmul start=True clears its whole PSUM bank, so each
            # concurrently-accumulating group needs its own bank/pool.
            psS = bctx.enter_context(tc.tile_pool(name="psS", bufs=3, space="PSUM"))
            psV0 = bctx.enter_context(tc.tile_pool(name="psV0", bufs=1, space="PSUM"))
            psV1 = bctx.enter_context(tc.tile_pool(name="psV1", bufs=1, space="PSUM"))
            psD = bctx.enter_context(tc.tile_pool(name="psD", bufs=1, space="PSUM"))
            psO = bctx.enter_context(tc.tile_pool(name="psO", bufs=2, space="PSUM"))
            expool = bctx.enter_context(tc.tile_pool(name="expp", bufs=4))
            rpool = bctx.enter_context(tc.tile_pool(name="rdp", bufs=2))
            opool = bctx.enter_context(tc.tile_pool(name="obp", bufs=2))
            apool = bctx.enter_context(tc.tile_pool(name="attp", bufs=1))
            attnT = apool.tile([P, 4, S], bf16)   # [d2, (h,dc), qpos]

            def oproj(sc):
                ob = opool.tile([P, HID], f32, tag="ob", name=f"ob{sc}")
                for n in range(4):
                    po = psO.tile([P, 512], f32, tag="po", name=f"po{sc}_{n}")
                    for t in range(4):
                        nc.tensor.matmul(
                            po[:], attnT[:, t, sc * P:(sc + 1) * P],
                            woT[:, t, n * 512:(n + 1) * 512],
                            start=(t == 0), stop=(t == 3))
                    if n % 2 == 0:
                        nc.scalar.copy(ob[:, n * 512:(n + 1) * 512], po[:])
                    else:
                        nc.vector.tensor_copy(out=ob[:, n * 512:(n + 1) * 512],
                                              in_=po[:])
                nc.sync.dma_start(out_d[sc * P:(sc + 1) * P, :], ob[:])

            pend_oproj = []
            for p in range(8):
                K = 2 * p + 2
                q0 = p * 256
                # diagonal (masked) blocks first: their extra DVE hop gets
                # hidden under the remaining kb's score matmuls
                order = [2 * p, 2 * p + 1] + list(range(2 * p))
                for h in range(2):
                    exp_tiles = {}
                    pv0 = psV0.tile([P, 256], f32, tag="pv0", name=f"pv0_{p}_{h}")
                    pv1 = psV1.tile([P, 256], f32, tag="pv1", name=f"pv1_{p}_{h}")
                    dnb = psD.tile([P, 256], f32, tag="dn", name=f"dn{p}_{h}")

                    def emit_sc(kb, p=p, h=h, q0=q0, exp_tiles=exp_tiles):
                        t = psS.tile([P, 256], f32, tag="sc",
                                     name=f"sc{p}_{h}_{kb}")
                        for dc in range(2):
                            nc.tensor.matmul(
                                t[:], kT[:, dc, kb * P:(kb + 1) * P],
                                qT[:, h, dc, q0:q0 + 256],
                                start=(dc == 0), stop=(dc == 1))
                        if kb == 2 * p:
                            nc.vector.tensor_add(t[:, 0:P], t[:, 0:P],
                                                 mdiagT[:])
                        elif kb == 2 * p + 1:
                            nc.vector.memset(t[:, 0:P], -1e9)
                            nc.vector.tensor_add(t[:, P:2 * P], t[:, P:2 * P],
                                                 mdiagT[:])
                        e = expool.tile([P, 256], bf16, tag="exp",
                                        name=f"ex{p}_{h}_{kb}")
                        nc.scalar.activation(e[:], t[:], ACT.Exp,
                                             bias=nshift[:])
                        exp_tiles[kb] = e

                    emit_sc(order[0])
                    # a deferred output projection fills the mask+exp latency
                    if pend_oproj:
                        oproj(pend_oproj.pop(0))
                    emit_sc(order[1])
                    for i, kb in enumerate(order):
                        e = exp_tiles.pop(kb)
                        st, sp = i == 0, i == K - 1
                        nc.tensor.matmul(pv0[:], v_sb[:, kb, 0:P],
                                         e[:], start=st, stop=sp)
                        nc.tensor.matmul(pv1[:], v_sb[:, kb, P:HD],
                                         e[:], start=st, stop=sp)
                        nc.tensor.matmul(dnb[:], ones_k[:, 0:P], e[:],
                                         start=st, stop=sp)
                        if i + 2 < K:
                            emit_sc(order[i + 2])
                    rd = rpool.tile([P, 256], f32, tag="rd", name=f"rd{p}_{h}")
                    nc.vector.reciprocal(rd[:], dnb[:])
                    if debug_dump:
                        nc.sync.dma_start(rd_dbg[p * 2 + h], rd[:])
                    nc.vector.tensor_mul(attnT[:, h * 2, q0:q0 + 256],
                                         pv0[:], rd[:])
                    nc.vector.tensor_mul(attnT[:, h * 2 + 1, q0:q0 + 256],
                                         pv1[:], rd[:])
                pend_oproj += [2 * p, 2 * p + 1]
            for sc in pend_oproj:
                oproj(sc)
            if debug_dump:
                nc.sync.dma_start(at_dbg.ap(), attnT[:])

    nc.compile()
    return nc


def prep_core_inputs(inputs, core, use_f32r=True):
    """Host-side sharding for one core. Returns the in_map dict."""
    cvt = to_f32r if use_f32r else (lambda a: np.asarray(a, np.float32))
    bf16np = mybir.dt.np(bf16)
    b, kv, qp = core // 4, (core % 4) // 2, core % 2
    hq0 = kv * 4 + qp * 2           # first of the two query heads
    hidden = np.asarray(inputs["hidden_states"], np.float32)
    cos = np.asarray(inputs["cos"], np.float32)
    sin = np.asarray(inputs["sin"], np.float32)
    Wq = np.asarray(inputs["Wq"], np.float32)
    Wk = np.asarray(inputs["Wk"], np.float32)
    Wv = np.asarray(inputs["Wv"], np.float32)
    Wo = np.asarray(inputs["Wo"], np.float32)
    qw = np.asarray(inputs["q_norm_w"], np.float32)
    kw = np.asarray(inputs["k_norm_w"], np.float32)

    hT = np.ascontiguousarray(hidden[b].T).reshape(NHC, P, S)
    Wq_c = Wq[hq0 * HD:(hq0 + 2) * HD]          # [512, HID]
    Wk_c = Wk[kv * HD:(kv + 1) * HD]            # [256, HID]
    Wv_c = Wv[kv * HD:(kv + 1) * HD]
    wT = np.ascontiguousarray(
        np.concatenate([Wq_c.T, Wk_c.T, Wv_c.T], axis=1)).reshape(NHC, P, 1024)

    def cs_pack(w, cb, sb):
        rot_w = np.concatenate([w[P:], w[:P]])   # w[(d+128)%256]
        cosw = cb * w[None, :]
        sinw = sb * rot_w[None, :]
        sinw[:, :P] *= -1.0
        return np.ascontiguousarray(
            np.concatenate([cosw, sinw], axis=1)).reshape(NSC, P, 2 * HD)

    csq = cs_pack(qw, cos[b], sin[b])
    csk = cs_pack(kw, cos[b], sin[b])
    woT = np.ascontiguousarray(
        Wo[:, hq0 * HD:(hq0 + 2) * HD].T).reshape(4, P, HID)
    return {"hT": cvt(hT), "wT": cvt(wT),
            "csq": csq.astype(np.float32), "csk": csk.astype(np.float32),
            "woT": woT.astype(bf16np)}


def mask_is_causal(mask):
    m = np.asarray(mask)
    tri = np.tril(np.ones((S, S), dtype=bool))
    for b in range(m.shape[0]):
        mb = m[b, 0]
        if not (mb[tri] == 0.0).all():
            return False
        if not (mb[~tri] <= -1e8).all():
            return False
    return True


def reference_numpy(inputs, f64=True):
    """Defensive fallback for non-causal masks (never hit in practice)."""
    dt = np.float64 if f64 else np.float32
    hs = np.asarray(inputs["hidden_states"], dt)
    cos = np.asarray(inputs["cos"], dt)
    sin = np.asarray(inputs["sin"], dt)
    mask = np.asarray(inputs["attention_mask"], dt)
    Wq, Wk, Wv, Wo = (np.asarray(inputs[k], dt)
                      for k in ("Wq", "Wk", "Wv", "Wo"))
    qw = np.asarray(inputs["q_norm_w"], dt)
    kw = np.asarray(inputs["k_norm_w"], dt)

    def rms(x, w):
        return x / np.sqrt((x * x).mean(-1, keepdims=True) + EPS) * w

    def rope(x, c, s):
        x1, x2 = x[..., :HD // 2], x[..., HD // 2:]
        rot = np.concatenate([-x2, x1], axis=-1)
        return x * c[:, :, None, :] + rot * s[:, :, None, :]

    b, s_, _ = hs.shape
    q = (hs @ Wq.T).reshape(b, s_, NH, HD)
    k = (hs @ Wk.T).reshape(b, s_, NKV, HD)
    v = (hs @ Wv.T).reshape(b, s_, NKV, HD)
    q = rope(rms(q, qw), cos, sin).transpose(0, 2, 1, 3)
    k = rope(rms(k, kw), cos, sin).transpose(0, 2, 1, 3)
    v = rms(v, 1.0).transpose(0, 2, 1, 3)
    k = np.repeat(k, NH // NKV, axis=1)
    v = np.repeat(v, NH // NKV, axis=1)
    sc = np.einsum("bhqd,bhkd->bhqk", q, k) + mask
    sc = sc - sc.max(-1, keepdims=True)
    p = np.exp(sc)
    p /= p.sum(-1, keepdims=True)
    o = np.einsum("bhqk,bhkd->bqhd", p, v).reshape(b, s_, NH * HD)
    return (o @ Wo.T).astype(np.float32)


_PROGRAM = {}


def get_program(use_f32r=True, debug_dump=False):
    key = (use_f32r, debug_dump)
    if key not in _PROGRAM:
        _PROGRAM[key] = build_program(use_f32r=use_f32r,
                                      debug_dump=debug_dump)
    return _PROGRAM[key]


def run_on_hw(inputs, use_f32r=True, trace=False, debug_dump=False, **kw):
    from concourse.bass_utils import run_bass_kernel_spmd

    nc = get_program(use_f32r=use_f32r, debug_dump=debug_dump)
    in_maps = [prep_core_inputs(inputs, c, use_f32r) for c in range(8)]
    br = run_bass_kernel_spmd(nc, in_maps, list(range(8)), trace=trace, **kw)
    out = np.empty((B, S, HID), np.float32)
    for b in range(B):
        out[b] = br.results[4 * b]["out"] + br.results[4 * b + 1]["out"] \
            + br.results[4 * b + 2]["out"] + br.results[4 * b + 3]["out"]
    return out, br


def kernel(**inputs):
    if not mask_is_causal(inputs["attention_mask"]):
        return reference_numpy(inputs)
    out, _ = run_on_hw(inputs, use_f32r=True, trace=False)
    return out
